# revision 9
# baseline (speedup 1.0000x reference)
"""Trainium2 Bass kernel for the 3-population LIF spiking network.

Math (per element e = (b, f), per population c with a_c = dt/tau_c, r_c = 1 - a_c):
    v_dec_t = r_c * v_{t-1} + a_c * x_t
    z_t     = [v_dec_t > vth_c]
    v_t     = v_dec_t * (1 - z_t)
    out_o   = sum_{c,t} conv_w[c] * lin_w[o,t] * z_{c,t} + conv_b * sum_t lin_w[o,t] + lin_b[o]

Device formulation uses a rescaled state  s_hat_t = g_t * v_t  with the forced
chain g_t = -g_{t-1} / r_c (g_0 = 1/a_c), which turns each step into exactly
two fused DVE ops plus one 2x-mode compare:
    update : s_hat_dec = (x_t * alpha_t) + s_hat          (scalar_tensor_tensor)
    compare: z = (s_hat_dec CMP theta_t)  -> fp16 {0,1}   (tensor_scalar, 2x)
    reset  : s_hat'   = (z - 1) * s_hat_dec               (scalar_tensor_tensor)
where CMP is is_gt for even t (g_t > 0) and is_lt for odd t (g_t < 0).
Output accumulation runs on the TensorEngine as fp16 diagonal matmuls into
PSUM (two accumulators, one per output channel), so the DVE only carries the
recurrence.  All input-dependent constants enter as tiny device tensors, so
the compiled NEFF depends on shapes only and is cacheable across calls.

Sharding: batch-parallel over 8 NeuronCores (32 batches/core), no collectives.
"""

import os
import sys

import numpy as np

for _p in ("/opt/trn_rl_repo", "/root/.axon_site/_ro/trn_rl_repo"):
    if os.path.isdir(_p) and _p not in sys.path:
        sys.path.append(_p)

import concourse.bass as bass
import concourse.bacc as bacc
import concourse.tile as tile
from concourse import mybir
from concourse.bass_utils import run_bass_kernel_spmd

F32 = mybir.dt.float32
F16 = mybir.dt.float16
OP = mybir.AluOpType

NCORES = 8
B, T, F = 256, 10, 16384
C = 3  # LIF populations
O = 2  # output channels
P = 128  # SBUF partitions

BC = B // NCORES  # batches per core
E = BC * F  # elements per core
NCOLS = E // P  # free-dim columns per core (4096)
MSLICE = 512  # matmul moving-operand slice (one PSUM bank)


def build_nc(ncols=NCOLS, w=1024, nrep=1):
    """Build the per-core Bass program. All cores run the identical program.

    nrep > 1 wraps the whole compute in an on-device repeat loop (benchmark
    builds only), so per-dispatch overhead amortizes out of timing runs.
    """
    assert ncols % w == 0
    nchunk = ncols // w
    msl = min(w, MSLICE)
    assert w % msl == 0
    nslice = w // msl
    from contextlib import ExitStack

    nc = bacc.Bacc("TRN2", target_bir_lowering=False, debug=False)
    x_in = nc.declare_dram_parameter("x", [T, P, ncols], F32, isOutput=False)
    alpha_in = nc.declare_dram_parameter("alpha", [P, C * T], F32, isOutput=False)
    theta_in = nc.declare_dram_parameter("theta", [P, C * T], F32, isOutput=False)
    avth_in = nc.declare_dram_parameter("avth", [P, 2 * C], F32, isOutput=False)
    diag_in = nc.declare_dram_parameter("diag", [P, C * T * O * P], F16, isOutput=False)
    cbias_in = nc.declare_dram_parameter("cbias", [P, O], F32, isOutput=False)
    y_out = nc.declare_dram_parameter("y", [O, P, ncols], F32, isOutput=True)

    with tile.TileContext(nc) as tc:
        with (
            tc.tile_pool(name="consts", bufs=1) as cpool,
            tc.tile_pool(name="xp", bufs=2) as xpool,
            tc.tile_pool(name="state", bufs=2) as spool,
            tc.tile_pool(name="zp", bufs=2) as zpool,
            tc.tile_pool(name="outp", bufs=2) as opool,
            tc.tile_pool(name="acc", bufs=2, space="PSUM") as ppool,
        ):
# Consts are DMA'd to staging tiles, then copied on the DVE. Compute
            # reads the copies, so downstream DVE ops order behind them on the
            # engine itself instead of carrying extra DMA-semaphore waits (the
            # 3-source TensorScalarPtr encoding has a single sync-wait slot).
            def staged_const(name, cols, dt_):
                stg = cpool.tile([P, cols], dt_, name=f"{name}_stg")
                nc.sync.dma_start(out=stg[:], in_=_cin[name][:])
                fin = cpool.tile([P, cols], dt_, name=f"{name}_sb")
                nc.vector.tensor_copy(fin[:], stg[:])
                return fin

            _cin = {"alpha": alpha_in, "theta": theta_in, "avth": avth_in,
                    "cbias": cbias_in, "diag": diag_in}
            alpha_sb = staged_const("alpha", C * T, F32)
            theta_sb = staged_const("theta", C * T, F32)
            avth_sb = staged_const("avth", 2 * C, F32)
            cbias_sb = staged_const("cbias", O, F32)
            diag_sb = staged_const("diag", C * T * O * P, F16)

            rep_ctx = ExitStack()
            if nrep > 1:
                rep_ctx.enter_context(tc.For_i(0, nrep, 1))
            for k in range(nchunk):
                sl = slice(k * w, (k + 1) * w)
                xs = []
                for t in range(T):
                    xt = xpool.tile([P, w], F32, name=f"x{t}", tag=f"x{t}")
                    nc.sync.dma_start(out=xt[:], in_=x_in[t, :, sl])
                    xs.append(xt)

                acc = [ppool.tile([P, w], F32, name=f"acc{o}", tag=f"acc{o}") for o in range(O)]

                s_cur = [None] * C
                for t in range(T):
                    for c in range(C):
                        j = c * T + t
                        z = zpool.tile([P, w], F16, name=f"z{c}", tag=f"z{c}")
                        if t == 0:
                            # z = (x * a_c) > vth_c  -- matches reference rounding
                            nc.vector.tensor_scalar(
                                z[:], xs[0][:],
                                avth_sb[:, c : c + 1], avth_sb[:, C + c : C + c + 1],
                                OP.mult, OP.is_gt,
                            )
                            sd = xs[0]
                        else:
                            sd = spool.tile([P, w], F32, name=f"sd{c}", tag=f"sd{c}")
                            nc.vector.scalar_tensor_tensor(
                                sd[:], xs[t][:], alpha_sb[:, j : j + 1], s_cur[c][:],
                                OP.mult, OP.add,
                            )
                            cmp = OP.is_gt if (t % 2 == 0) else OP.is_lt
                            nc.vector.tensor_scalar(
                                z[:], sd[:], theta_sb[:, j : j + 1], None, cmp,
                            )
                        for o in range(O):
                            dcol = (j * O + o) * P
                            for s in range(nslice):
                                ssl = slice(s * msl, (s + 1) * msl)
                                nc.tensor.matmul(
                                    acc[o][:, ssl],
                                    lhsT=diag_sb[:, dcol : dcol + P],
                                    rhs=z[:, ssl],
                                    start=(t == 0 and c == 0),
                                    stop=(t == T - 1 and c == C - 1),
                                )
                        if t < T - 1:
                            s_new = spool.tile([P, w], F32, name=f"s{c}", tag=f"s{c}")
                            nc.vector.scalar_tensor_tensor(
                                s_new[:], z[:], 1.0, sd[:],
                                OP.subtract, OP.mult,
                            )
                            s_cur[c] = s_new

                for o in range(O):
                    ot = opool.tile([P, w], F32, name=f"out{o}", tag=f"out{o}")
                    nc.scalar.activation(
                        ot[:], acc[o][:],
                        mybir.ActivationFunctionType.Identity,
                        bias=cbias_sb[:, o : o + 1], scale=1.0,
                    )
                    nc.sync.dma_start(out=y_out[o, :, sl], in_=ot[:])
            rep_ctx.close()
    nc.finalize()
    return nc


def make_consts(tau, vth, conv_w, conv_b, lin_w, lin_b):
    """Host-side constant tensors. tau/vth: [C] f32; conv_w: [C]; lin_w: [O, T]."""
    # Reference computes a = DT * tau_inv with DT a weak python float -> f32.
    a32 = np.float32(0.001) * tau.astype(np.float32)  # [C] f32
    a64 = a32.astype(np.float64)
    r64 = 1.0 - a64
    assert np.all((a64 > 0) & (a64 < 1)), "kernel assumes 0 < dt/tau < 1"

    alpha = np.zeros((C, T), np.float32)
    theta = np.zeros((C, T), np.float32)
    g = 1.0 / a64  # g_0
    for t in range(T):
        if t > 0:
            g = -g / r64
            alpha[:, t] = (g * a64).astype(np.float32)
        theta[:, t] = (g * vth.astype(np.float64)).astype(np.float32)

    avth = np.concatenate([a32, vth.astype(np.float32)])  # [2C]

    cw64 = conv_w.reshape(-1).astype(np.float64)  # [C]
    lw64 = lin_w.astype(np.float64)  # [O, T]
    coef = np.einsum("c,ot->cto", cw64, lw64).astype(np.float16)  # [C, T, O]
    ident = np.eye(P, dtype=np.float16)
    # diag layout: [P, (c,t,o)-major, P] flattened on free dim
    diag = np.zeros((P, C * T * O, P), np.float16)
    for c in range(C):
        for t in range(T):
            for o in range(O):
                diag[:, (c * T + t) * O + o, :] = coef[c, t, o] * ident
    diag = diag.reshape(P, C * T * O * P)

    cbias = (
        np.float64(conv_b.reshape(-1)[0]) * lw64.sum(axis=1)
        + lin_b.astype(np.float64)
    ).astype(np.float32)  # [O]

    rep = lambda v: np.repeat(v.reshape(1, -1), P, axis=0).astype(v.dtype)
    return {
        "alpha": np.ascontiguousarray(rep(alpha.reshape(-1))),
        "theta": np.ascontiguousarray(rep(theta.reshape(-1))),
        "avth": np.ascontiguousarray(rep(avth)),
        "diag": np.ascontiguousarray(diag),
        "cbias": np.ascontiguousarray(rep(cbias)),
    }


_NC_CACHE = {}


def get_nc(w=1024):
    key = (NCOLS, w)
    if key not in _NC_CACHE:
        _NC_CACHE[key] = build_nc(ncols=NCOLS, w=w)
    return _NC_CACHE[key]


def prepare_in_maps(inputs, tau1, tau2, tau3, vth1, vth2, vth3,
                    conv_w, conv_b, lin_w, lin_b):
    inputs = np.asarray(inputs, dtype=np.float32)
    assert inputs.shape == (B, T, F), inputs.shape

    tau = np.array([tau1[0], tau2[0], tau3[0]], np.float32)
    vth = np.array([vth1[0], vth2[0], vth3[0]], np.float32)
    consts = make_consts(tau, vth, np.asarray(conv_w), np.asarray(conv_b),
                         np.asarray(lin_w), np.asarray(lin_b))

    in_maps = []
    for i in range(NCORES):
        xc = inputs[i * BC : (i + 1) * BC]  # [BC, T, F]
        xdev = np.ascontiguousarray(xc.transpose(1, 0, 2).reshape(T, P, NCOLS))
        in_maps.append({"x": xdev, **consts})
    return in_maps


def gather_out(results):
    out = np.empty((B, O, F), np.float32)
    for i in range(NCORES):
        y = np.asarray(results[i]["y"])  # [O, P, NCOLS]
        out[i * BC : (i + 1) * BC] = y.reshape(O, BC, F).transpose(1, 0, 2)
    return out


def kernel(inputs, tau1, tau2, tau3, vth1, vth2, vth3, conv_w, conv_b, lin_w, lin_b):
    in_maps = prepare_in_maps(inputs, tau1, tau2, tau3, vth1, vth2, vth3,
                              conv_w, conv_b, lin_w, lin_b)
    res = run_bass_kernel_spmd(get_nc(), in_maps, list(range(NCORES)))
    return gather_out(res.results)


# revision 13
# speedup vs baseline: 1.6229x; 1.6229x over previous
"""Trainium2 Bass kernel for the 3-population LIF spiking network.

Math (per element e = (b, f), per population c with a_c = dt/tau_c, r_c = 1 - a_c):
    v_dec_t = r_c * v_{t-1} + a_c * x_t
    z_t     = [v_dec_t > vth_c]
    v_t     = v_dec_t * (1 - z_t)
    out_o   = sum_{c,t} conv_w[c] * lin_w[o,t] * z_{c,t} + conv_b * sum_t lin_w[o,t] + lin_b[o]

Device formulation uses a rescaled state  s_hat_t = g_t * v_t  with the forced
chain g_t = -g_{t-1} / r_c (g_0 = 1/a_c), which turns each step into exactly
two fused DVE ops plus one 2x-mode compare:
    update : s_hat_dec = (x_t * alpha_t) + s_hat          (scalar_tensor_tensor)
    compare: z = (s_hat_dec CMP theta_t)  -> fp16 {0,1}   (tensor_scalar, 2x)
    reset  : s_hat'   = (z - 1) * s_hat_dec               (scalar_tensor_tensor)
where CMP is is_gt for even t (g_t > 0) and is_lt for odd t (g_t < 0).
Output accumulation runs on the TensorEngine as fp16 diagonal matmuls into
PSUM (two accumulators, one per output channel), so the DVE only carries the
recurrence.  All input-dependent constants enter as tiny device tensors, so
the compiled NEFF depends on shapes only and is cacheable across calls.

Sharding: batch-parallel over 8 NeuronCores (32 batches/core), no collectives.
"""

import os
import sys

import numpy as np

for _p in ("/opt/trn_rl_repo", "/root/.axon_site/_ro/trn_rl_repo"):
    if os.path.isdir(_p) and _p not in sys.path:
        sys.path.append(_p)

import concourse.bass as bass
import concourse.bacc as bacc
import concourse.tile as tile
from concourse import mybir
from concourse.bass_utils import run_bass_kernel_spmd

F32 = mybir.dt.float32
F16 = mybir.dt.float16
OP = mybir.AluOpType

NCORES = 8
B, T, F = 256, 10, 16384
C = 3  # LIF populations
O = 2  # output channels
P = 128  # SBUF partitions

BC = B // NCORES  # batches per core
E = BC * F  # elements per core
NCOLS = E // P  # free-dim columns per core (4096)
MSLICE = 512  # matmul moving-operand slice (one PSUM bank)


def build_nc(ncols=NCOLS, w=1024, nrep=1):
    """Build the per-core Bass program. All cores run the identical program.

    nrep > 1 wraps the whole compute in an on-device repeat loop (benchmark
    builds only), so per-dispatch overhead amortizes out of timing runs.
    """
    assert ncols % w == 0
    nchunk = ncols // w
    msl = min(w, MSLICE)
    assert w % msl == 0
    nslice = w // msl
    from contextlib import ExitStack

    nc = bacc.Bacc("TRN2", target_bir_lowering=False, debug=False)
    x_in = nc.declare_dram_parameter("x", [T, P, ncols], F32, isOutput=False)
    alpha_in = nc.declare_dram_parameter("alpha", [P, C * T], F32, isOutput=False)
    theta_in = nc.declare_dram_parameter("theta", [P, C * T], F32, isOutput=False)
    avth_in = nc.declare_dram_parameter("avth", [P, 2 * C], F32, isOutput=False)
    diag_in = nc.declare_dram_parameter("diag", [P, C * T * O * P], F16, isOutput=False)
    cbias_in = nc.declare_dram_parameter("cbias", [P, O], F32, isOutput=False)
    y_out = nc.declare_dram_parameter("y", [O, P, ncols], F32, isOutput=True)

    with tile.TileContext(nc) as tc:
        with (
            tc.tile_pool(name="consts", bufs=1) as cpool,
            tc.tile_pool(name="xp", bufs=2) as xpool,
            tc.tile_pool(name="state", bufs=2) as spool,
            tc.tile_pool(name="zp", bufs=2) as zpool,
            tc.tile_pool(name="outp", bufs=2) as opool,
            tc.tile_pool(name="acc", bufs=2, space="PSUM") as ppool,
        ):
# Consts are DMA'd to staging tiles, then copied on the DVE. Compute
            # reads the copies, so downstream DVE ops order behind them on the
            # engine itself instead of carrying extra DMA-semaphore waits (the
            # 3-source TensorScalarPtr encoding has a single sync-wait slot).
            def staged_const(name, cols, dt_):
                stg = cpool.tile([P, cols], dt_, name=f"{name}_stg")
                nc.sync.dma_start(out=stg[:], in_=_cin[name][:])
                fin = cpool.tile([P, cols], dt_, name=f"{name}_sb")
                nc.vector.tensor_copy(fin[:], stg[:])
                return fin

            _cin = {"alpha": alpha_in, "theta": theta_in, "avth": avth_in,
                    "cbias": cbias_in, "diag": diag_in}
            alpha_sb = staged_const("alpha", C * T, F32)
            theta_sb = staged_const("theta", C * T, F32)
            avth_sb = staged_const("avth", 2 * C, F32)
            cbias_sb = staged_const("cbias", O, F32)
            diag_sb = staged_const("diag", C * T * O * P, F16)

            rep_ctx = ExitStack()
            if nrep > 1:
                rep_ctx.enter_context(tc.For_i(0, nrep, 1))
            for k in range(nchunk):
                sl = slice(k * w, (k + 1) * w)
                xs = []
                for t in range(T):
                    xt = xpool.tile([P, w], F32, name=f"x{t}", tag=f"x{t}")
                    nc.sync.dma_start(out=xt[:], in_=x_in[t, :, sl])
                    xs.append(xt)

                acc = [ppool.tile([P, w], F32, name=f"acc{o}", tag=f"acc{o}") for o in range(O)]

                s_cur = [None] * C
                for t in range(T):
                    for c in range(C):
                        j = c * T + t
                        # (walrus rejects TensorScalarPtr on Pool, so all
                        # state ops stay on the DVE; compares live on ACT)
                        eng = nc.vector
                        z = zpool.tile([P, w], F16, name=f"z{c}", tag=f"z{c}")
                        if t == 0:
                            # w = Sign(a_c * x - vth_c) in {-1, +1}
                            nc.scalar.activation(
                                z[:], xs[0][:],
                                mybir.ActivationFunctionType.Sign,
                                bias=avth_sb[:, C + c : C + c + 1],
                                scale=avth_sb[:, c : c + 1],
                            )
                            sd = xs[0]
                        else:
                            sd = spool.tile([P, w], F32, name=f"sd{c}", tag=f"sd{c}")
                            eng.scalar_tensor_tensor(
                                sd[:], xs[t][:], alpha_sb[:, j : j + 1], s_cur[c][:],
                                OP.mult, OP.add,
                            )
                            # w = Sign(sign_t * (s_dec - theta)) in {-1, +1}
                            nc.scalar.activation(
                                z[:], sd[:],
                                mybir.ActivationFunctionType.Sign,
                                bias=theta_sb[:, j : j + 1],
                                scale=(1.0 if t % 2 == 0 else -1.0),
                            )
                        for o in range(O):
                            dcol = (j * O + o) * P
                            for s in range(nslice):
                                ssl = slice(s * msl, (s + 1) * msl)
                                nc.tensor.matmul(
                                    acc[o][:, ssl],
                                    lhsT=diag_sb[:, dcol : dcol + P],
                                    rhs=z[:, ssl],
                                    start=(t == 0 and c == 0),
                                    stop=(t == T - 1 and c == C - 1),
                                )
                        if t < T - 1:
                            s_new = spool.tile([P, w], F32, name=f"s{c}", tag=f"s{c}")
                            eng.scalar_tensor_tensor(
                                s_new[:], z[:], 1.0, sd[:],
                                OP.subtract, OP.mult,
                            )
                            s_cur[c] = s_new

                for o in range(O):
                    ot = opool.tile([P, w], F32, name=f"out{o}", tag=f"out{o}")
                    nc.scalar.activation(
                        ot[:], acc[o][:],
                        mybir.ActivationFunctionType.Identity,
                        bias=cbias_sb[:, o : o + 1], scale=1.0,
                    )
                    nc.sync.dma_start(out=y_out[o, :, sl], in_=ot[:])
            rep_ctx.close()
    nc.finalize()
    return nc


def make_consts(tau, vth, conv_w, conv_b, lin_w, lin_b):
    """Host-side constant tensors. tau/vth: [C] f32; conv_w: [C]; lin_w: [O, T]."""
    # Reference computes a = DT * tau_inv with DT a weak python float -> f32.
    a32 = np.float32(0.001) * tau.astype(np.float32)  # [C] f32
    a64 = a32.astype(np.float64)
    r64 = 1.0 - a64
    assert np.all((a64 > 0) & (a64 < 1)), "kernel assumes 0 < dt/tau < 1"

    # Spikes are carried as w = Sign(.) in {-1, +1}; the reset op computes
    # (w - 1) * s_dec in {-2*s_dec, 0}, so the state scale chain picks up an
    # exact factor of -2 each step: g_{t+1} = -2 * g_t / r, g_0 = 1/a.
    alpha = np.zeros((C, T), np.float32)
    theta = np.zeros((C, T), np.float32)  # ACT bias: -sign_t * g_t * vth
    g = 1.0 / a64  # g_0
    for t in range(T):
        if t > 0:
            g = -2.0 * g / r64
            alpha[:, t] = (g * a64).astype(np.float32)
        sign_t = 1.0 if t % 2 == 0 else -1.0
        theta[:, t] = (-sign_t * g * vth.astype(np.float64)).astype(np.float32)

    avth = np.concatenate([a32, -vth.astype(np.float32)])  # [a_c | -vth_c]

    cw64 = conv_w.reshape(-1).astype(np.float64)  # [C]
    lw64 = lin_w.astype(np.float64)  # [O, T]
    # z = (w + 1)/2  =>  sum coef*z = sum (coef/2)*w + sum coef/2
    coef = np.einsum("c,ot->cto", cw64, lw64)  # [C, T, O]
    half = (coef / 2.0).astype(np.float16)
    ident = np.eye(P, dtype=np.float16)
    diag = np.zeros((P, C * T * O, P), np.float16)
    for c in range(C):
        for t in range(T):
            for o in range(O):
                diag[:, (c * T + t) * O + o, :] = half[c, t, o] * ident
    diag = diag.reshape(P, C * T * O * P)

    cbias = (
        np.float64(conv_b.reshape(-1)[0]) * lw64.sum(axis=1)
        + lin_b.astype(np.float64)
        + half.astype(np.float64).sum(axis=(0, 1))  # sum_ct coef_cto/2 (fp16 vals)
    ).astype(np.float32)  # [O]

    rep = lambda v: np.repeat(v.reshape(1, -1), P, axis=0).astype(v.dtype)
    return {
        "alpha": np.ascontiguousarray(rep(alpha.reshape(-1))),
        "theta": np.ascontiguousarray(rep(theta.reshape(-1))),
        "avth": np.ascontiguousarray(rep(avth)),
        "diag": np.ascontiguousarray(diag),
        "cbias": np.ascontiguousarray(rep(cbias)),
    }


_NC_CACHE = {}


def get_nc(w=1024):
    key = (NCOLS, w)
    if key not in _NC_CACHE:
        _NC_CACHE[key] = build_nc(ncols=NCOLS, w=w)
    return _NC_CACHE[key]


def prepare_in_maps(inputs, tau1, tau2, tau3, vth1, vth2, vth3,
                    conv_w, conv_b, lin_w, lin_b):
    inputs = np.asarray(inputs, dtype=np.float32)
    assert inputs.shape == (B, T, F), inputs.shape

    tau = np.array([tau1[0], tau2[0], tau3[0]], np.float32)
    vth = np.array([vth1[0], vth2[0], vth3[0]], np.float32)
    consts = make_consts(tau, vth, np.asarray(conv_w), np.asarray(conv_b),
                         np.asarray(lin_w), np.asarray(lin_b))

    in_maps = []
    for i in range(NCORES):
        xc = inputs[i * BC : (i + 1) * BC]  # [BC, T, F]
        xdev = np.ascontiguousarray(xc.transpose(1, 0, 2).reshape(T, P, NCOLS))
        in_maps.append({"x": xdev, **consts})
    return in_maps


def gather_out(results):
    out = np.empty((B, O, F), np.float32)
    for i in range(NCORES):
        y = np.asarray(results[i]["y"])  # [O, P, NCOLS]
        out[i * BC : (i + 1) * BC] = y.reshape(O, BC, F).transpose(1, 0, 2)
    return out


def kernel(inputs, tau1, tau2, tau3, vth1, vth2, vth3, conv_w, conv_b, lin_w, lin_b):
    in_maps = prepare_in_maps(inputs, tau1, tau2, tau3, vth1, vth2, vth3,
                              conv_w, conv_b, lin_w, lin_b)
    res = run_bass_kernel_spmd(get_nc(), in_maps, list(range(NCORES)))
    return gather_out(res.results)
